# revision 7
# baseline (speedup 1.0000x reference)
"""Trainium2 Bass kernel for nn_Attention_25701084299349.

Reference computation (per batch sample b, with C=256, CQK=64, hw=4096):
    Q = w_src  @ x_src + b_src          # (CQK, hw)   1x1 conv
    K = w_ref  @ x_ref + b_ref          # (CQK, hw)
    G = w_gate @ x_ref + b_gate         # (C, hw)
    E[i, j]  = sum_k Q[k, i] K[k, j]    # (hw, hw)
    A        = softmax(E / 16, axis=j)
    out[c,i] = sum_j A[i, j] G[c, j]
    final    = gamma * out + x_src

Sharding: 8 cores = 4 batch samples x 2 halves of the query (i) axis.
Each core computes K and G for its full sample (duplicated across the
2 cores of a sample) and the E/softmax/AV pipeline for its 2048 rows.

On-chip layout (per core):
  - All matmul inputs in bf16; PSUM accumulation in fp32.
  - E is computed transposed, E_T[j, i] (j on partitions), so the exp'd
    attention tiles are directly usable as the AV matmul's moving operand
    and the softmax denominator sum over j becomes a partition reduction
    done with an all-ones 128x128 matmul (which also broadcasts it).
  - Softmax normalization, gamma, b_gate and the residual add are folded
    into a small per-(c,i) epilogue after the AV matmul.
  - No max-subtraction in softmax: |E/16| < ~0.4 for these inputs.
"""

import sys

for _p in ("/opt/trn_rl_repo",):
    if _p not in sys.path:
        sys.path.append(_p)

import ml_dtypes
import numpy as np

import concourse.bass as bass
import concourse.tile as tile
from concourse import bacc, mybir
from concourse.bass_utils import run_bass_kernel_spmd

B, C, CQK = 4, 256, 64
HW = 4096          # h * w
HALF = HW // 2     # i-range per core
KT = C // 128      # 2 contraction tiles for the 1x1 convs
IB = 512           # i-block size
NBLK = HALF // IB  # 4 i-blocks
NJT = HW // 128    # 32 j-tiles
GJT = 4            # j-tiles per exp group
NGRP = NJT // GJT  # 8 groups per i-block
SCALE = 1.0 / 16.0  # C ** -0.5

F32 = mybir.dt.float32
BF16 = mybir.dt.bfloat16
AF = mybir.ActivationFunctionType

_CACHE = {}


def _build():
    nc = bacc.Bacc("TRN2", target_bir_lowering=False, debug=False)

    d_xsrc32 = nc.dram_tensor("xsrc32", [C, HALF], F32, kind="ExternalInput").ap()
    d_xsrc16 = nc.dram_tensor("xsrc16", [C, HALF], BF16, kind="ExternalInput").ap()
    d_xref16 = nc.dram_tensor("xref16", [C, HW], BF16, kind="ExternalInput").ap()
    d_wsrcT = nc.dram_tensor("wsrcT", [C, CQK], BF16, kind="ExternalInput").ap()
    d_wrefT = nc.dram_tensor("wrefT", [C, CQK], BF16, kind="ExternalInput").ap()
    d_wgateT = nc.dram_tensor("wgateT", [C, C], BF16, kind="ExternalInput").ap()
    d_bsrc = nc.dram_tensor("bsrc", [CQK, 1], F32, kind="ExternalInput").ap()
    d_bref = nc.dram_tensor("bref", [CQK, 1], F32, kind="ExternalInput").ap()
    d_gb = nc.dram_tensor("gb", [C, 1], F32, kind="ExternalInput").ap()
    d_gamma = nc.dram_tensor("gammav", [128, 1], F32, kind="ExternalInput").ap()
    d_ones = nc.dram_tensor("ones", [128, 128], BF16, kind="ExternalInput").ap()
    d_out = nc.dram_tensor("out", [C, HALF], F32, kind="ExternalOutput").ap()

    with tile.TileContext(nc) as tc:
        _frees = []

        def ptile(shape, dtype, name):
            t, free = tc.tile(shape, dtype, name=name)
            _frees.append(free)
            return t

        # ---- persistent SBUF tensors ----
        s_wsrcT = ptile([128, KT, CQK], BF16, "s_wsrcT")
        s_wrefT = ptile([128, KT, CQK], BF16, "s_wrefT")
        s_wgateT = ptile([128, KT, C], BF16, "s_wgateT")
        s_bsrc = ptile([128, 1], F32, "s_bsrc")
        s_bref = ptile([128, 1], F32, "s_bref")
        s_gb = ptile([128, 2], F32, "s_gb")
        s_gamma = ptile([128, 1], F32, "s_gamma")
        s_ones = ptile([128, 128], BF16, "s_ones")
        s_xsrc32 = ptile([128, KT, HALF], F32, "s_xsrc32")
        s_xsrc16 = ptile([128, KT, HALF], BF16, "s_xsrc16")
        s_xref16 = ptile([128, KT, HW], BF16, "s_xref16")
        s_q2 = ptile([128, HALF], BF16, "s_q2")    # Q duplicated on both partition halves
        s_k2 = ptile([128, HW], BF16, "s_k2")      # K duplicated on both partition halves
        s_gate = ptile([128, NJT * C], BF16, "s_gate")  # gate_T, [j, jt*C + c]

        nc.sync.dma_start(out=s_wsrcT, in_=d_wsrcT.rearrange("(a p) m -> p a m", p=128))
        nc.sync.dma_start(out=s_wrefT, in_=d_wrefT.rearrange("(a p) m -> p a m", p=128))
        nc.sync.dma_start(out=s_wgateT, in_=d_wgateT.rearrange("(a p) m -> p a m", p=128))
        nc.sync.dma_start(out=s_bsrc[0:CQK, :], in_=d_bsrc)
        nc.sync.dma_start(out=s_bref[0:CQK, :], in_=d_bref)
        nc.sync.dma_start(out=s_gb, in_=d_gb.rearrange("(a p) m -> p (a m)", p=128))
        nc.sync.dma_start(out=s_gamma, in_=d_gamma)
        nc.sync.dma_start(out=s_ones, in_=d_ones)
        nc.sync.dma_start(out=s_xsrc32, in_=d_xsrc32.rearrange("(a p) m -> p a m", p=128))
        nc.sync.dma_start(out=s_xsrc16, in_=d_xsrc16.rearrange("(a p) m -> p a m", p=128))
        nc.sync.dma_start(out=s_xref16, in_=d_xref16.rearrange("(a p) m -> p a m", p=128))

        # ---- Q / K projections ----
        with tc.tile_pool(name="qk_ps", bufs=2, space="PSUM") as qk_pool:
            qp = qk_pool.tile([CQK, HALF], F32, name="qp", tag="qk")
            for it in range(HALF // IB):
                for kt in range(KT):
                    nc.tensor.matmul(
                        qp[:, it * IB:(it + 1) * IB],
                        lhsT=s_wsrcT[:, kt, :],
                        rhs=s_xsrc16[:, kt, it * IB:(it + 1) * IB],
                        start=(kt == 0),
                        stop=(kt == KT - 1),
                    )
            nc.scalar.activation(
                out=s_q2[0:CQK, :], in_=qp[:], func=AF.Identity, bias=s_bsrc[0:CQK, 0:1]
            )
            for h in range(2):
                kp = qk_pool.tile([CQK, HALF], F32, name=f"kp{h}", tag="qk")
                for it in range(HALF // IB):
                    for kt in range(KT):
                        nc.tensor.matmul(
                            kp[:, it * IB:(it + 1) * IB],
                            lhsT=s_wrefT[:, kt, :],
                            rhs=s_xref16[:, kt, h * HALF + it * IB:h * HALF + (it + 1) * IB],
                            start=(kt == 0),
                            stop=(kt == KT - 1),
                        )
                nc.scalar.activation(
                    out=s_k2[0:CQK, h * HALF:(h + 1) * HALF], in_=kp[:],
                    func=AF.Identity, bias=s_bref[0:CQK, 0:1],
                )
            nc.sync.dma_start(out=s_q2[CQK:128, :], in_=s_q2[0:CQK, :])
            nc.sync.dma_start(out=s_k2[CQK:128, :], in_=s_k2[0:CQK, :])

        # ---- pools for the main pipeline ----
        e_pool = tc.alloc_tile_pool(name="e_ps", bufs=1, space="PSUM")
        a_pool = tc.alloc_tile_pool(name="a_sb", bufs=16)
        gs_pool = tc.alloc_tile_pool(name="gs_sb", bufs=2)
        r_pool = None
        av_pool = None
        rs_pool = tc.alloc_tile_pool(name="rs_sb", bufs=3)
        ep_pool = tc.alloc_tile_pool(name="ep_sb", bufs=2)
        out_pool = tc.alloc_tile_pool(name="out_sb", bufs=4)

        a_tiles = [[None] * NGRP for _ in range(NBLK)]
        r_tiles = [None] * NBLK
        av_tiles = [None] * NBLK

        def energy_group(m, g):
            """E matmuls + exp + partial softmax sums for (block m, group g)."""
            ep = e_pool.tile([128, GJT * IB], F32, name=f"ep_{m}_{g}", tag="ep")
            for s in range(GJT):
                jt = g * GJT + s
                bp = (s % 2) * CQK  # alternate partition halves -> PE row packing
                nc.tensor.matmul(
                    ep[:, s * IB:(s + 1) * IB],
                    lhsT=s_k2[bp:bp + CQK, jt * 128:(jt + 1) * 128],
                    rhs=s_q2[bp:bp + CQK, m * IB:(m + 1) * IB],
                    start=True,
                    stop=True,
                )
            a_t = a_pool.tile([128, GJT * IB], BF16, name=f"a_{m}_{g}", tag="a")
            nc.scalar.activation(out=a_t[:], in_=ep[:], func=AF.Exp, scale=SCALE)
            a_tiles[m][g] = a_t
            t0 = gs_pool.tile([128, IB], BF16, name=f"t0_{m}_{g}", tag="t0")
            t1 = gs_pool.tile([128, IB], BF16, name=f"t1_{m}_{g}", tag="t1")
            gs = gs_pool.tile([128, IB], BF16, name=f"gs_{m}_{g}", tag=f"gs{g}")
            nc.vector.tensor_add(t0, a_t[:, 0 * IB:1 * IB], a_t[:, 1 * IB:2 * IB])
            nc.vector.tensor_add(t1, a_t[:, 2 * IB:3 * IB], a_t[:, 3 * IB:4 * IB])
            nc.vector.tensor_add(gs, t0, t1)
            return gs

        def sum_stage(m, gsums):
            """Combine group sums, partition-reduce+broadcast via ones-matmul,
            reciprocal, fold gamma."""
            cur = list(gsums)
            lvl = 0
            while len(cur) > 1:
                nxt = []
                for i in range(0, len(cur), 2):
                    o = gs_pool.tile([128, IB], BF16, name=f"cmb_{m}_{lvl}_{i}",
                                     tag=f"cmb{lvl}_{i}")
                    nc.vector.tensor_add(o, cur[i], cur[i + 1])
                    nxt.append(o)
                cur = nxt
                lvl += 1
            rp = r_pool.tile([128, IB], F32, name=f"rp_{m}", tag="rp")
            nc.tensor.matmul(rp[:], lhsT=s_ones[:], rhs=cur[0][:], start=True, stop=True)
            rs = rs_pool.tile([128, IB], F32, name=f"rs_{m}", tag="rs")
            nc.vector.reciprocal(rs, rp[:])
            rs2 = rs_pool.tile([128, IB], F32, name=f"rs2_{m}", tag="rs2")
            nc.vector.tensor_scalar_mul(rs2, rs, s_gamma[:, 0:1])
            r_tiles[m] = rs2

        def av_group(m, g):
            """AV matmuls for block m, k-tiles g*GJT .. g*GJT+GJT-1."""
            for s in range(GJT):
                jt = g * GJT + s
                for ct in range(2):
                    nc.tensor.matmul(
                        av_tiles[m][ct][:],
                        lhsT=s_gate[:, jt * C + ct * 128:jt * C + (ct + 1) * 128],
                        rhs=a_tiles[m][jt // GJT][:, (jt % GJT) * IB:(jt % GJT + 1) * IB],
                        start=(jt == 0),
                        stop=(jt == NJT - 1),
                    )

        def epilogue(m):
            """final = gamma*(av/sumexp + b_gate) + x_src, then DMA out."""
            for ct in range(2):
                t = ep_pool.tile([128, IB], F32, name=f"t_{m}_{ct}", tag="ept")
                nc.vector.tensor_mul(t, av_tiles[m][ct][:], r_tiles[m])
                t2 = ep_pool.tile([128, IB], F32, name=f"u_{m}_{ct}", tag="ept2")
                nc.scalar.activation(out=t2[:], in_=t[:], func=AF.Identity,
                                     bias=s_gb[:, ct:ct + 1])
                fin = out_pool.tile([128, IB], F32, name=f"f_{m}_{ct}", tag="fin")
                nc.vector.tensor_add(fin, t2, s_xsrc32[:, ct, m * IB:(m + 1) * IB])
                nc.sync.dma_start(
                    out=d_out[ct * 128:(ct + 1) * 128, m * IB:(m + 1) * IB], in_=fin
                )

        # ---- iteration 0: energy/exp for block 0, gate projection in gaps ----
        with tc.tile_pool(name="g_ps", bufs=4, space="PSUM") as g_pool:
            gsums = []
            for g in range(NGRP):
                gsums.append(energy_group(0, g))
                for s in range(GJT):
                    jt = g * GJT + s
                    gp = g_pool.tile([128, C], F32, name=f"gp_{jt}", tag="gp")
                    for kt in range(KT):
                        nc.tensor.matmul(
                            gp[:],
                            lhsT=s_xref16[:, kt, jt * 128:(jt + 1) * 128],
                            rhs=s_wgateT[:, kt, :],
                            start=(kt == 0),
                            stop=(kt == KT - 1),
                        )
                    nc.vector.tensor_copy(s_gate[:, jt * C:(jt + 1) * C], gp[:])

        r_pool = tc.alloc_tile_pool(name="r_ps", bufs=2, space="PSUM")
        av_pool = tc.alloc_tile_pool(name="av_ps", bufs=1, space="PSUM")
        sum_stage(0, gsums)

        # ---- iterations 1..NBLK: E/exp(m) interleaved with AV(m-1) ----
        for m in range(1, NBLK + 1):
            av_tiles[m - 1] = [
                av_pool.tile([128, IB], F32, name=f"av_{m - 1}_{ct}", tag=f"av{ct}")
                for ct in range(2)
            ]
            gsums = []
            for g in range(NGRP):
                if m < NBLK:
                    gsums.append(energy_group(m, g))
                av_group(m - 1, g)
            if m < NBLK:
                sum_stage(m, gsums)
            epilogue(m - 1)

        # release in reverse allocation (stack) order
        for p in (av_pool, r_pool, out_pool, ep_pool, rs_pool, gs_pool, a_pool,
                  e_pool):
            p.release()
        for free in reversed(_frees):
            free()

    nc.compile()
    return nc


def _get_nc():
    if "nc" not in _CACHE:
        _CACHE["nc"] = _build()
    return _CACHE["nc"]


def kernel(**inputs):
    src = np.asarray(inputs["source_features"], dtype=np.float32)
    ref = np.asarray(inputs["reference_features"], dtype=np.float32)
    w_src = np.asarray(inputs["w_src"], dtype=np.float32)
    b_src = np.asarray(inputs["b_src"], dtype=np.float32)
    w_ref = np.asarray(inputs["w_ref"], dtype=np.float32)
    b_ref = np.asarray(inputs["b_ref"], dtype=np.float32)
    w_gate = np.asarray(inputs["w_gate"], dtype=np.float32)
    b_gate = np.asarray(inputs["b_gate"], dtype=np.float32)
    gamma = np.asarray(inputs["gamma"], dtype=np.float32)

    bf = ml_dtypes.bfloat16
    wsrcT = np.ascontiguousarray(w_src.T).astype(bf)
    wrefT = np.ascontiguousarray(w_ref.T).astype(bf)
    wgateT = np.ascontiguousarray(w_gate.T).astype(bf)
    bsrc = np.ascontiguousarray(b_src.reshape(CQK, 1))
    bref = np.ascontiguousarray(b_ref.reshape(CQK, 1))
    gb = np.ascontiguousarray((gamma[0] * b_gate).reshape(C, 1)).astype(np.float32)
    gammav = np.full((128, 1), gamma[0], dtype=np.float32)
    ones = np.ones((128, 128), dtype=bf)

    in_maps = []
    for k in range(8):
        b, h = divmod(k, 2)
        xsrc32 = np.ascontiguousarray(
            src[b].reshape(C, HW)[:, h * HALF:(h + 1) * HALF]
        )
        in_maps.append({
            "xsrc32": xsrc32,
            "xsrc16": xsrc32.astype(bf),
            "xref16": ref[b].reshape(C, HW).astype(bf),
            "wsrcT": wsrcT,
            "wrefT": wrefT,
            "wgateT": wgateT,
            "bsrc": bsrc,
            "bref": bref,
            "gb": gb,
            "gammav": gammav,
            "ones": ones,
        })

    nc = _get_nc()
    res = run_bass_kernel_spmd(nc, in_maps, core_ids=list(range(8)))

    out = np.empty((B, C, HW), dtype=np.float32)
    for k in range(8):
        b, h = divmod(k, 2)
        out[b, :, h * HALF:(h + 1) * HALF] = res.results[k]["out"]
    return out.reshape(B, C, 64, 64)


# revision 8
# speedup vs baseline: 64.4758x; 64.4758x over previous
"""Trainium2 Bass kernel for nn_Attention_25701084299349.

Reference computation (per batch sample b, with C=256, CQK=64, hw=4096):
    Q = w_src  @ x_src + b_src          # (CQK, hw)   1x1 conv
    K = w_ref  @ x_ref + b_ref          # (CQK, hw)
    G = w_gate @ x_ref + b_gate         # (C, hw)
    E[i, j]  = sum_k Q[k, i] K[k, j]    # (hw, hw)
    A        = softmax(E / 16, axis=j)
    out[c,i] = sum_j A[i, j] G[c, j]
    final    = gamma * out + x_src

Sharding: 8 cores = 4 batch samples x 2 halves of the query (i) axis.
Each core computes K and G for its full sample (duplicated across the
2 cores of a sample) and the E/softmax/AV pipeline for its 2048 rows.

On-chip layout (per core):
  - All matmul inputs in bf16; PSUM accumulation in fp32.
  - E is computed transposed, E_T[j, i] (j on partitions), so the exp'd
    attention tiles are directly usable as the AV matmul's moving operand
    and the softmax denominator sum over j becomes a partition reduction
    done with an all-ones 128x128 matmul (which also broadcasts it).
  - Softmax normalization, gamma, b_gate and the residual add are folded
    into a small per-(c,i) epilogue after the AV matmul.
  - No max-subtraction in softmax: |E/16| < ~0.4 for these inputs.
"""

import sys

for _p in ("/opt/trn_rl_repo",):
    if _p not in sys.path:
        sys.path.append(_p)

import ml_dtypes
import numpy as np

import concourse.bass as bass
import concourse.tile as tile
from concourse import bacc, mybir
from concourse.bass_utils import run_bass_kernel_spmd

B, C, CQK = 4, 256, 64
HW = 4096          # h * w
HALF = HW // 2     # i-range per core
KT = C // 128      # 2 contraction tiles for the 1x1 convs
IB = 512           # i-block size
NBLK = HALF // IB  # 4 i-blocks
NJT = HW // 128    # 32 j-tiles
GJT = 4            # j-tiles per exp group
NGRP = NJT // GJT  # 8 groups per i-block
SCALE = 1.0 / 16.0  # C ** -0.5

F32 = mybir.dt.float32
BF16 = mybir.dt.bfloat16
AF = mybir.ActivationFunctionType

_CACHE = {}


def _build(reps=1):
    nc = bacc.Bacc("TRN2", target_bir_lowering=False, debug=False)

    d_xsrc32 = nc.dram_tensor("xsrc32", [C, HALF], F32, kind="ExternalInput").ap()
    d_xsrc16 = nc.dram_tensor("xsrc16", [C, HALF], BF16, kind="ExternalInput").ap()
    d_xref16 = nc.dram_tensor("xref16", [C, HW], BF16, kind="ExternalInput").ap()
    d_wsrcT = nc.dram_tensor("wsrcT", [C, CQK], BF16, kind="ExternalInput").ap()
    d_wrefT = nc.dram_tensor("wrefT", [C, CQK], BF16, kind="ExternalInput").ap()
    d_wgateT = nc.dram_tensor("wgateT", [C, C], BF16, kind="ExternalInput").ap()
    d_bsrc = nc.dram_tensor("bsrc", [CQK, 1], F32, kind="ExternalInput").ap()
    d_bref = nc.dram_tensor("bref", [CQK, 1], F32, kind="ExternalInput").ap()
    d_gb = nc.dram_tensor("gb", [C, 1], F32, kind="ExternalInput").ap()
    d_gamma = nc.dram_tensor("gammav", [128, 1], F32, kind="ExternalInput").ap()
    d_ones = nc.dram_tensor("ones", [128, 128], BF16, kind="ExternalInput").ap()
    d_out = nc.dram_tensor("out", [C, HALF], F32, kind="ExternalOutput").ap()

    with tile.TileContext(nc) as tc:
      for _rep in range(reps):
        _frees = []

        def ptile(shape, dtype, name):
            t, free = tc.tile(shape, dtype, name=name)
            _frees.append(free)
            return t

        # ---- persistent SBUF tensors ----
        s_wsrcT = ptile([128, KT, CQK], BF16, "s_wsrcT")
        s_wrefT = ptile([128, KT, CQK], BF16, "s_wrefT")
        s_wgateT = ptile([128, KT, C], BF16, "s_wgateT")
        s_bsrc = ptile([128, 1], F32, "s_bsrc")
        s_bref = ptile([128, 1], F32, "s_bref")
        s_gb = ptile([128, 2], F32, "s_gb")
        s_gamma = ptile([128, 1], F32, "s_gamma")
        s_ones = ptile([128, 128], BF16, "s_ones")
        s_xsrc32 = ptile([128, KT, HALF], F32, "s_xsrc32")
        s_xsrc16 = ptile([128, KT, HALF], BF16, "s_xsrc16")
        s_xref16 = ptile([128, KT, HW], BF16, "s_xref16")
        s_q2 = ptile([128, HALF], BF16, "s_q2")    # Q duplicated on both partition halves
        s_k2 = ptile([128, HW], BF16, "s_k2")      # K duplicated on both partition halves
        s_gate = ptile([128, NJT * C], BF16, "s_gate")  # gate_T, [j, jt*C + c]

        nc.sync.dma_start(out=s_wsrcT, in_=d_wsrcT.rearrange("(a p) m -> p a m", p=128))
        nc.sync.dma_start(out=s_wrefT, in_=d_wrefT.rearrange("(a p) m -> p a m", p=128))
        nc.sync.dma_start(out=s_wgateT, in_=d_wgateT.rearrange("(a p) m -> p a m", p=128))
        nc.sync.dma_start(out=s_bsrc[0:CQK, :], in_=d_bsrc)
        nc.sync.dma_start(out=s_bref[0:CQK, :], in_=d_bref)
        nc.sync.dma_start(out=s_gb, in_=d_gb.rearrange("(a p) m -> p (a m)", p=128))
        nc.sync.dma_start(out=s_gamma, in_=d_gamma)
        nc.sync.dma_start(out=s_ones, in_=d_ones)
        nc.sync.dma_start(out=s_xsrc32, in_=d_xsrc32.rearrange("(a p) m -> p a m", p=128))
        nc.sync.dma_start(out=s_xsrc16, in_=d_xsrc16.rearrange("(a p) m -> p a m", p=128))
        nc.sync.dma_start(out=s_xref16, in_=d_xref16.rearrange("(a p) m -> p a m", p=128))

        # ---- Q / K projections ----
        with tc.tile_pool(name="qk_ps", bufs=2, space="PSUM") as qk_pool:
            qp = qk_pool.tile([CQK, HALF], F32, name="qp", tag="qk")
            for it in range(HALF // IB):
                for kt in range(KT):
                    nc.tensor.matmul(
                        qp[:, it * IB:(it + 1) * IB],
                        lhsT=s_wsrcT[:, kt, :],
                        rhs=s_xsrc16[:, kt, it * IB:(it + 1) * IB],
                        start=(kt == 0),
                        stop=(kt == KT - 1),
                    )
            nc.scalar.activation(
                out=s_q2[0:CQK, :], in_=qp[:], func=AF.Identity, bias=s_bsrc[0:CQK, 0:1]
            )
            for h in range(2):
                kp = qk_pool.tile([CQK, HALF], F32, name=f"kp{h}", tag="qk")
                for it in range(HALF // IB):
                    for kt in range(KT):
                        nc.tensor.matmul(
                            kp[:, it * IB:(it + 1) * IB],
                            lhsT=s_wrefT[:, kt, :],
                            rhs=s_xref16[:, kt, h * HALF + it * IB:h * HALF + (it + 1) * IB],
                            start=(kt == 0),
                            stop=(kt == KT - 1),
                        )
                nc.scalar.activation(
                    out=s_k2[0:CQK, h * HALF:(h + 1) * HALF], in_=kp[:],
                    func=AF.Identity, bias=s_bref[0:CQK, 0:1],
                )
            nc.sync.dma_start(out=s_q2[CQK:128, :], in_=s_q2[0:CQK, :])
            nc.sync.dma_start(out=s_k2[CQK:128, :], in_=s_k2[0:CQK, :])

        # ---- pools for the main pipeline ----
        e_pool = tc.alloc_tile_pool(name="e_ps", bufs=1, space="PSUM")
        a_pool = tc.alloc_tile_pool(name="a_sb", bufs=16)
        gs_pool = tc.alloc_tile_pool(name="gs_sb", bufs=2)
        r_pool = None
        av_pool = None
        rs_pool = tc.alloc_tile_pool(name="rs_sb", bufs=3)
        ep_pool = tc.alloc_tile_pool(name="ep_sb", bufs=2)
        out_pool = tc.alloc_tile_pool(name="out_sb", bufs=4)

        a_tiles = [[None] * NGRP for _ in range(NBLK)]
        r_tiles = [None] * NBLK
        av_tiles = [None] * NBLK

        def energy_group(m, g):
            """E matmuls + exp + partial softmax sums for (block m, group g)."""
            ep = e_pool.tile([128, GJT * IB], F32, name=f"ep_{m}_{g}", tag="ep")
            for s in range(GJT):
                jt = g * GJT + s
                bp = (s % 2) * CQK  # alternate partition halves -> PE row packing
                nc.tensor.matmul(
                    ep[:, s * IB:(s + 1) * IB],
                    lhsT=s_k2[bp:bp + CQK, jt * 128:(jt + 1) * 128],
                    rhs=s_q2[bp:bp + CQK, m * IB:(m + 1) * IB],
                    start=True,
                    stop=True,
                )
            a_t = a_pool.tile([128, GJT * IB], BF16, name=f"a_{m}_{g}", tag="a")
            nc.scalar.activation(out=a_t[:], in_=ep[:], func=AF.Exp, scale=SCALE)
            a_tiles[m][g] = a_t
            t0 = gs_pool.tile([128, IB], BF16, name=f"t0_{m}_{g}", tag="t0")
            t1 = gs_pool.tile([128, IB], BF16, name=f"t1_{m}_{g}", tag="t1")
            gs = gs_pool.tile([128, IB], BF16, name=f"gs_{m}_{g}", tag=f"gs{g}")
            nc.vector.tensor_add(t0, a_t[:, 0 * IB:1 * IB], a_t[:, 1 * IB:2 * IB])
            nc.vector.tensor_add(t1, a_t[:, 2 * IB:3 * IB], a_t[:, 3 * IB:4 * IB])
            nc.vector.tensor_add(gs, t0, t1)
            return gs

        def sum_stage(m, gsums):
            """Combine group sums, partition-reduce+broadcast via ones-matmul,
            reciprocal, fold gamma."""
            cur = list(gsums)
            lvl = 0
            while len(cur) > 1:
                nxt = []
                for i in range(0, len(cur), 2):
                    o = gs_pool.tile([128, IB], BF16, name=f"cmb_{m}_{lvl}_{i}",
                                     tag=f"cmb{lvl}_{i}")
                    nc.vector.tensor_add(o, cur[i], cur[i + 1])
                    nxt.append(o)
                cur = nxt
                lvl += 1
            rp = r_pool.tile([128, IB], F32, name=f"rp_{m}", tag="rp")
            nc.tensor.matmul(rp[:], lhsT=s_ones[:], rhs=cur[0][:], start=True, stop=True)
            rs = rs_pool.tile([128, IB], F32, name=f"rs_{m}", tag="rs")
            nc.vector.reciprocal(rs, rp[:])
            rs2 = rs_pool.tile([128, IB], F32, name=f"rs2_{m}", tag="rs2")
            nc.vector.tensor_scalar_mul(rs2, rs, s_gamma[:, 0:1])
            r_tiles[m] = rs2

        def av_group(m, g):
            """AV matmuls for block m, k-tiles g*GJT .. g*GJT+GJT-1."""
            for s in range(GJT):
                jt = g * GJT + s
                for ct in range(2):
                    nc.tensor.matmul(
                        av_tiles[m][ct][:],
                        lhsT=s_gate[:, jt * C + ct * 128:jt * C + (ct + 1) * 128],
                        rhs=a_tiles[m][jt // GJT][:, (jt % GJT) * IB:(jt % GJT + 1) * IB],
                        start=(jt == 0),
                        stop=(jt == NJT - 1),
                    )

        def epilogue(m):
            """final = gamma*(av/sumexp + b_gate) + x_src, then DMA out."""
            for ct in range(2):
                t = ep_pool.tile([128, IB], F32, name=f"t_{m}_{ct}", tag="ept")
                nc.vector.tensor_mul(t, av_tiles[m][ct][:], r_tiles[m])
                t2 = ep_pool.tile([128, IB], F32, name=f"u_{m}_{ct}", tag="ept2")
                nc.scalar.activation(out=t2[:], in_=t[:], func=AF.Identity,
                                     bias=s_gb[:, ct:ct + 1])
                fin = out_pool.tile([128, IB], F32, name=f"f_{m}_{ct}", tag="fin")
                nc.vector.tensor_add(fin, t2, s_xsrc32[:, ct, m * IB:(m + 1) * IB])
                nc.sync.dma_start(
                    out=d_out[ct * 128:(ct + 1) * 128, m * IB:(m + 1) * IB], in_=fin
                )

        # ---- iteration 0: energy/exp for block 0, gate projection in gaps ----
        with tc.tile_pool(name="g_ps", bufs=4, space="PSUM") as g_pool:
            gsums = []
            for g in range(NGRP):
                gsums.append(energy_group(0, g))
                for s in range(GJT):
                    jt = g * GJT + s
                    gp = g_pool.tile([128, C], F32, name=f"gp_{jt}", tag="gp")
                    for kt in range(KT):
                        nc.tensor.matmul(
                            gp[:],
                            lhsT=s_xref16[:, kt, jt * 128:(jt + 1) * 128],
                            rhs=s_wgateT[:, kt, :],
                            start=(kt == 0),
                            stop=(kt == KT - 1),
                        )
                    nc.vector.tensor_copy(s_gate[:, jt * C:(jt + 1) * C], gp[:])

        r_pool = tc.alloc_tile_pool(name="r_ps", bufs=2, space="PSUM")
        av_pool = tc.alloc_tile_pool(name="av_ps", bufs=1, space="PSUM")
        sum_stage(0, gsums)

        # ---- iterations 1..NBLK: E/exp(m) interleaved with AV(m-1) ----
        for m in range(1, NBLK + 1):
            av_tiles[m - 1] = [
                av_pool.tile([128, IB], F32, name=f"av_{m - 1}_{ct}", tag=f"av{ct}")
                for ct in range(2)
            ]
            gsums = []
            for g in range(NGRP):
                if m < NBLK:
                    gsums.append(energy_group(m, g))
                av_group(m - 1, g)
            if m < NBLK:
                sum_stage(m, gsums)
            epilogue(m - 1)

        # release in reverse allocation (stack) order
        for p in (av_pool, r_pool, out_pool, ep_pool, rs_pool, gs_pool, a_pool,
                  e_pool):
            p.release()
        for free in reversed(_frees):
            free()

    nc.compile()
    return nc


def _get_nc():
    if "nc" not in _CACHE:
        _CACHE["nc"] = _build()
    return _CACHE["nc"]


def kernel(**inputs):
    src = np.asarray(inputs["source_features"], dtype=np.float32)
    ref = np.asarray(inputs["reference_features"], dtype=np.float32)
    w_src = np.asarray(inputs["w_src"], dtype=np.float32)
    b_src = np.asarray(inputs["b_src"], dtype=np.float32)
    w_ref = np.asarray(inputs["w_ref"], dtype=np.float32)
    b_ref = np.asarray(inputs["b_ref"], dtype=np.float32)
    w_gate = np.asarray(inputs["w_gate"], dtype=np.float32)
    b_gate = np.asarray(inputs["b_gate"], dtype=np.float32)
    gamma = np.asarray(inputs["gamma"], dtype=np.float32)

    bf = ml_dtypes.bfloat16
    wsrcT = np.ascontiguousarray(w_src.T).astype(bf)
    wrefT = np.ascontiguousarray(w_ref.T).astype(bf)
    wgateT = np.ascontiguousarray(w_gate.T).astype(bf)
    bsrc = np.ascontiguousarray(b_src.reshape(CQK, 1))
    bref = np.ascontiguousarray(b_ref.reshape(CQK, 1))
    gb = np.ascontiguousarray((gamma[0] * b_gate).reshape(C, 1)).astype(np.float32)
    gammav = np.full((128, 1), gamma[0], dtype=np.float32)
    ones = np.ones((128, 128), dtype=bf)

    in_maps = []
    for k in range(8):
        b, h = divmod(k, 2)
        xsrc32 = np.ascontiguousarray(
            src[b].reshape(C, HW)[:, h * HALF:(h + 1) * HALF]
        )
        in_maps.append({
            "xsrc32": xsrc32,
            "xsrc16": xsrc32.astype(bf),
            "xref16": ref[b].reshape(C, HW).astype(bf),
            "wsrcT": wsrcT,
            "wrefT": wrefT,
            "wgateT": wgateT,
            "bsrc": bsrc,
            "bref": bref,
            "gb": gb,
            "gammav": gammav,
            "ones": ones,
        })

    nc = _get_nc()
    res = run_bass_kernel_spmd(nc, in_maps, core_ids=list(range(8)))

    out = np.empty((B, C, HW), dtype=np.float32)
    for k in range(8):
        b, h = divmod(k, 2)
        out[b, :, h * HALF:(h + 1) * HALF] = res.results[k]["out"]
    return out.reshape(B, C, 64, 64)
